# revision 11
# baseline (speedup 1.0000x reference)
"""Causal self-attention (RoPE, 16 heads, B=2 T=2048 C=1024) on 8 TRN2 cores.

Sharding: core = b*4 + g  (b = batch, g = head-group of 4 heads).
Each core computes the qkv projection for its 4 heads, RoPE, causal flash
attention, and the w_proj partial product for its head slice; the host sums
the 4 partials per batch.

Attention loop is (query-slice j, head-pair) so the output projection for
query block j=0 interleaves into j=1's attention as PE filler work (keeps
the HAM clock-gate warm and removes the serial tail).  Each head pair's
y accumulates packed in one PSUM tile [128,1024] (even head rows 0:64,
odd 64:128) matching the yT layout; softmax denominators accumulate on
DVE/GpSimd (dacc += px) and are reduced/broadcast with small f32r
matmuls on the PE.
"""

import numpy as np

# Problem constants (hardcoded per harness contract).
B = 2
T = 2048
C = 1024
N_HEAD = 16
HD = 64
HPC = 4           # heads per core
N_CORES = 8
ROPE_BASE = 10000.0
TS = 512          # qkv t-slice width

DTCFG = "bbbb"    # kept for test.py compat; kernel is all-bf16

_CACHE = {}


def _chunks512(off, end):
    """Split [off, end) on the 512 grid (PSUM bank alignment)."""
    out = []
    lo = off
    while lo < end:
        hi = min(end, (lo // 512 + 1) * 512)
        out.append((lo, hi))
        lo = hi
    return out


def _np_dt(ch):
    if ch == "b":
        import ml_dtypes
        return np.dtype(ml_dtypes.bfloat16)
    return np.dtype(np.float32)


def _build(t_len=T, dtcfg=None, debug=False):
    import concourse.tile as tile
    from concourse import bacc, mybir

    F32 = mybir.dt.float32
    F32R = mybir.dt.float32r
    BF16 = mybir.dt.bfloat16
    D_QKV = D_S = D_Y = D_P = BF16

    n_ts = t_len // TS          # qkv t-slices
    n_tt = t_len // 128         # 128-row t-tiles
    n_j = t_len // 1024         # attention 1024-wide tq slices

    nc = bacc.Bacc(None, target_bir_lowering=False, debug=False)
    with tile.TileContext(nc) as tc:
        with tc.tile_pool(name="dram", bufs=1, space="DRAM") as dram:
            xT = dram.tile([C, t_len], D_QKV, kind="ExternalInput")
            wqk = dram.tile([C, 8 * HD], D_QKV, kind="ExternalInput")
            wv = dram.tile([C, 4 * HD], D_QKV, kind="ExternalInput")
            wo = dram.tile([4 * HD, C], D_P, kind="ExternalInput")
            cost = dram.tile([128, t_len], F32, kind="ExternalInput")
            ssin = dram.tile([128, t_len], F32, kind="ExternalInput")
            utri = dram.tile([128, 128], D_Y, kind="ExternalInput")
            out = dram.tile([t_len, C], F32, kind="ExternalOutput")

            xT_c = xT.rearrange("(a p) t -> a p t", p=128)    # [8, 128, T]
            wqk_c = wqk.rearrange("(a p) m -> a p m", p=128)  # [8, 128, 512]
            wv_c = wv.rearrange("(a p) m -> a p m", p=128)    # [8, 128, 256]
            wo_c = wo.rearrange("(a p) m -> a p m", p=128)    # [2, 128, 1024]

            with (
                tc.tile_pool(name="persist", bufs=1) as persist,
                tc.tile_pool(name="qkT_pool", bufs=1) as qkT_pool,
            ):
                # Persistent tiles
                utri_sb = persist.tile([128, 128], D_Y)
                qkT = [qkT_pool.tile([128, t_len], D_S, name=f"qkT{m}")
                       for m in range(4)]
                # v layout [128, n_tt, HPC, HD]
                vext_sb = persist.tile([128, n_tt * HPC * HD], D_Y)
                vext_v = vext_sb.rearrange("p (i h d) -> p i h d", i=n_tt, d=HD)
                yT = [persist.tile([128, t_len], D_P, name=f"yT{k}")
                      for k in range(2)]
                wo_sb = [persist.tile([128, C], D_P, name=f"wo{k}")
                         for k in range(2)]
                # all-ones stationary: reduces dacc over partitions AND
                # broadcasts the denominator to 64 rows in one matmul
                ones_dn = persist.tile([128, 64], F32)
                nc.gpsimd.memset(ones_dn[:], 1.0)

                # ---------------- qkv phase ----------------
                with (
                    tc.tile_pool(name="wq_pool", bufs=1) as wq_pool,
                    tc.tile_pool(name="tab_pool", bufs=1) as tab_pool,
                    tc.tile_pool(name="xt_pool", bufs=1) as xt_pool,
                    tc.tile_pool(name="rope_pool", bufs=2) as rope_pool,
                    tc.tile_pool(name="acc_ps_pool", bufs=6,
                                 space="PSUM") as acc_ps_pool,
                ):
                    cos_sb = tab_pool.tile([128, t_len], F32)
                    ssin_sb = tab_pool.tile([128, t_len], F32)
                    wqk_sb = [wq_pool.tile([128, 8 * HD], D_QKV, name=f"wqk{c}")
                              for c in range(8)]
                    wv_sb = [wq_pool.tile([128, 4 * HD], D_QKV, name=f"wv{c}")
                             for c in range(8)]
                    # weights on the Scalar HWDGE queue (parallel to Sync, which
                    # is busy streaming xT)
                    for c in range(8):
                        nc.scalar.dma_start(out=wqk_sb[c], in_=wqk_c[c])
                    for c in range(8):
                        nc.scalar.dma_start(out=wv_sb[c], in_=wv_c[c])
                    for k in range(2):
                        nc.scalar.dma_start(out=wo_sb[k], in_=wo_c[k])

                    def rope(qkps, m, t0, width):
                        """RoPE a projected q/k PSUM tile into qkT[m]."""
                        qksb = rope_pool.tile([128, TS], F32, tag="qksb",
                                              name=f"qksb_{m}_{t0}")
                        nc.scalar.copy(out=qksb[:, :width], in_=qkps[:, :width])
                        # head dims host-permuted (evens | odds): rotate-half
                        # pair swap is a 32-partition block swap
                        swap = rope_pool.tile([128, TS], F32, tag="swap",
                                              name=f"swap_{m}_{t0}")
                        for hb in (0, 64):
                            nc.sync.dma_start(
                                out=swap[hb:hb + 32, :width],
                                in_=qksb[hb + 32:hb + 64, :width])
                            nc.sync.dma_start(
                                out=swap[hb + 32:hb + 64, :width],
                                in_=qksb[hb:hb + 32, :width])
                        tmp1 = rope_pool.tile([128, TS], F32, tag="tmp1",
                                              name=f"tmp1_{m}_{t0}")
                        nc.vector.tensor_mul(tmp1[:, :width], qkps[:, :width],
                                             cos_sb[:, t0:t0 + width])
                        tmp2 = rope_pool.tile([128, TS], F32, tag="tmp2",
                                              name=f"tmp2_{m}_{t0}")
                        nc.gpsimd.tensor_mul(tmp2[:, :width], swap[:, :width],
                                             ssin_sb[:, t0:t0 + width])
                        nc.vector.tensor_add(qkT[m][:, t0:t0 + width],
                                             tmp1[:, :width], tmp2[:, :width])

                    xT_sb = [xt_pool.tile([128, t_len], D_QKV,
                                          name=f"xTsb{c}") for c in range(8)]
                    # 512-wide rounds: the first qk accumulation group only
                    # needs cols 0:512 of every chunk, so it unblocks after
                    # round 0 instead of after half the stream
                    for qi in range(4):
                        for c in range(8):
                            nc.sync.dma_start(
                                out=xT_sb[c][:, qi * TS:(qi + 1) * TS],
                                in_=xT_c[c, :, qi * TS:(qi + 1) * TS])
                    nc.sync.dma_start(out=cos_sb, in_=cost[:])
                    nc.sync.dma_start(out=ssin_sb, in_=ssin[:])
                    nc.sync.dma_start(out=utri_sb, in_=utri[:])
                    # q/k: stationary w chunk streams all n_ts t-slices
                    # (weight load amortized n_ts x)
                    for m in (2, 3, 0, 1):      # k01 k23 q01 q23
                        qkps = [acc_ps_pool.tile([128, TS], F32, tag="acc",
                                                name=f"qkps_{m}_{ts}")
                                for ts in range(n_ts)]
                        for c in range(8):
                            for ts in range(n_ts):
                                nc.tensor.matmul(
                                    out=qkps[ts][:],
                                    lhsT=wqk_sb[c][:, m * 128:(m + 1) * 128],
                                    rhs=xT_sb[c][:, ts * TS:(ts + 1) * TS],
                                    start=(c == 0), stop=(c == 7),
                                )
                        for ts in range(n_ts):
                            rope(qkps[ts], m, ts * TS, TS)
                    # v after q/k so its stationary-switching matmuls run
                    # with the HAM clock-gate already warm: lhsT = xT chunk (stationary
                    # switches every matmul; no reuse available)
                    for i in range(n_tt):
                        vps = acc_ps_pool.tile([128, 4 * HD], F32,
                                               tag="acc", name=f"vps_{i}")
                        for c in range(8):
                            nc.tensor.matmul(
                                out=vps[:],
                                lhsT=xT_sb[c][:, i * 128:(i + 1) * 128],
                                rhs=wv_sb[c][:],
                                start=(c == 0), stop=(c == 7),
                            )
                        nc.vector.tensor_copy(
                            out=vext_v[:, i, :, :],
                            in_=vps.rearrange("p (h d) -> p h d", d=HD),
                        )

                # ---------------- attention + norm + proj ----------------
                with (
                    tc.tile_pool(name="p_pool", bufs=6) as p_pool,
                    tc.tile_pool(name="dacc_pool", bufs=2) as dacc_pool,
                    tc.tile_pool(name="rr_pool", bufs=2) as rr_pool,
                    tc.tile_pool(name="osb_pool", bufs=4) as osb_pool,
                    tc.tile_pool(name="ps_sx", bufs=2, space="PSUM") as ps_sx,
                    tc.tile_pool(name="ps_y", bufs=1, space="PSUM") as ps_y,
                    tc.tile_pool(name="ps_sm", bufs=2, space="PSUM") as ps_sm,
                ):
                    state = {}
                    proj_ready = []
                    proj_done = []

                    def emit_proj(budget):
                        while proj_ready and budget > 0:
                            tt, cs = proj_ready.pop(0)
                            ops = ps_sm.tile([128, 512], F32, tag="sm",
                                             name=f"ops_{tt}_{cs}")
                            for k in range(2):
                                nc.tensor.matmul(
                                    out=ops[:],
                                    lhsT=yT[k][:, tt * 128:(tt + 1) * 128],
                                    rhs=wo_sb[k][:, cs * 512:(cs + 1) * 512],
                                    start=(k == 0), stop=(k == 1),
                                )
                            osb = osb_pool.tile([128, 512], F32, tag="osb",
                                                name=f"osb_{tt}_{cs}")
                            nc.vector.tensor_copy(out=osb, in_=ops[:])
                            nc.sync.dma_start(
                                out=out[tt * 128:(tt + 1) * 128,
                                        cs * 512:(cs + 1) * 512],
                                in_=osb,
                            )
                            proj_done.append((tt, cs))
                            budget -= 1

                    def emit_s(pair, h, i, j):
                        """Scores + exp + mask + denominator accumulate for
                        (head h of pair, key tile i, query slice j)."""
                        base = 1024 * j
                        qtile, ktile = qkT[pair], qkT[2 + pair]
                        hoff = 64 * (h % 2)
                        c0 = max(base, 128 * i)
                        off = c0 - base
                        ch = _chunks512(off, 1024)
                        sx = ps_sx.tile([128, 1024], F32, tag="sps",
                                        name=f"sps_{pair}_{h}_{j}_{i}")
                        for (lo, hi) in ch:
                            nc.tensor.matmul(
                                out=sx[:, lo:hi],
                                lhsT=ktile[hoff:hoff + 64,
                                           128 * i:128 * (i + 1)],
                                rhs=qtile[hoff:hoff + 64,
                                          base + lo:base + hi],
                                start=True, stop=True,
                            )
                        px = p_pool.tile([128, 1024], D_Y, tag="psb",
                                         name=f"psb_{pair}_{h}_{j}_{i}")
                        nc.scalar.activation(
                            out=px[:, off:], in_=sx[:, off:],
                            func=mybir.ActivationFunctionType.Exp,
                        )
                        if i >= 8 * j:
                            nc.vector.tensor_mul(
                                px[:, off:off + 128],
                                px[:, off:off + 128],
                                utri_sb,
                            )
                        # denominator accumulate: dacc[:, h, :] += px
                        dacc_v = state["dacc_v"]
                        eng = nc.vector if (h % 2 == 0) else nc.gpsimd
                        if i == 0:
                            eng.tensor_copy(out=dacc_v[:, h % 2, :],
                                            in_=px[:, :])
                        else:
                            eng.tensor_add(dacc_v[:, h % 2, off:],
                                           dacc_v[:, h % 2, off:],
                                           px[:, off:])
                        state[("pend", h % 2)].append((i, px, ch))

                    def emit_y(pair, h, j):
                        i, px, ch = state[("pend", h % 2)].pop(0)
                        yps = state["yps"]
                        hoff = 64 * (h % 2)
                        base_v = (i * HPC + h) * HD
                        # reversed: unmasked chunks first (the masked
                        # diagonal block is in the first chunk)
                        for (lo, hi) in reversed(ch):
                            stop_i = 8 * j + (3 if lo < 512 else 7)
                            nc.tensor.matmul(
                                out=yps[hoff:hoff + 64, lo:hi],
                                lhsT=vext_sb[:, base_v:base_v + HD],
                                rhs=px[:, lo:hi],
                                start=(i == 0), stop=(i == stop_i),
                            )

                    def norm(pair, j, cs):
                        """Reduce denominators (broadcast via ones
                        stationary), reciprocal, and scale yps into yT for
                        query half cs of slice j."""
                        base = 1024 * j + 512 * cs
                        dacc_v, yps = state["dacc_v"], state["yps"]
                        dnb = ps_sm.tile([128, 512], F32, tag="sm",
                                         name=f"dn_{pair}_{j}_{cs}")
                        for hh in range(2):
                            nc.tensor.matmul(
                                out=dnb[64 * hh:64 * hh + 64, :],
                                lhsT=ones_dn[:],
                                rhs=dacc_v[:, hh, 512 * cs:512 * (cs + 1)],
                                start=True, stop=True,
                            )
                        rb = rr_pool.tile([128, 512], F32, tag="rr",
                                          name=f"rr_{pair}_{j}_{cs}")
                        nc.vector.reciprocal_approx_fast(out=rb, in_=dnb[:])
                        nc.vector.tensor_mul(
                            yT[pair][:, base:base + 512],
                            yps[:, 512 * cs:512 * (cs + 1)],
                            rb,
                        )

                    for j in range(n_j):
                        for pair in range(2):
                            heads = (2 * pair, 2 * pair + 1)
                            n_i = 8 * j + 8
                            state["yps"] = ps_y.tile(
                                [128, 1024], F32, tag="yps",
                                name=f"yps_{pair}_{j}")
                            dacc = dacc_pool.tile([128, 2048], F32, tag="dacc",
                                                  name=f"dacc_{pair}_{j}")
                            state["dacc_v"] = dacc.rearrange(
                                "p (h q) -> p h q", h=2)
                            state[("pend", 0)] = []
                            state[("pend", 1)] = []

                            for h in heads:
                                emit_s(pair, h, 0, j)
                            for i in range(1, n_i):
                                for h in heads:
                                    emit_s(pair, h, i, j)
                                for h in heads:
                                    emit_y(pair, h, j)
                                if i - 1 == 8 * j + 3:
                                    norm(pair, j, 0)
                                    if j == 1 and pair == 1:
                                        proj_ready.extend(
                                            (tt, cs) for tt in range(8, 12)
                                            for cs in range(2))
                                if j == 1:
                                    emit_proj(2)
                            for h in heads:
                                emit_y(pair, h, j)
                            norm(pair, j, 1)
                        if j == 0:
                            proj_ready.extend(
                                (tt, cs) for tt in range(8) for cs in range(2))
                        else:
                            proj_ready.extend(
                                (tt, cs) for tt in range(12, 16)
                                for cs in range(2))
                    emit_proj(len(proj_ready))
    nc.compile()
    names = dict(
        xT=xT.name, wqk=wqk.name, wv=wv.name, wo=wo.name,
        cost=cost.name, ssin=ssin.name, utri=utri.name,
        out=out.name,
    )
    return nc, names


# Head-dim permutation: evens first, odds last — turns the interleaved
# rotate-half pair swap into a contiguous 32-row block swap on device.
PERM = np.concatenate([np.arange(0, HD, 2), np.arange(1, HD, 2)])


def _host_constants(t_len=T, dtcfg=None):
    inv_freq = 1.0 / (ROPE_BASE ** (np.arange(0, HD, 2, dtype=np.float64) / HD))
    t = np.arange(t_len, dtype=np.float64)
    freqs = np.outer(t, inv_freq)                      # [T, 32]
    emb = np.concatenate([freqs, freqs], axis=-1)      # [T, 64]
    cosT = np.cos(emb).T.astype(np.float32)            # [64, T]
    sinT = np.sin(emb).T.astype(np.float32)
    sgn = np.where(np.arange(HD) % 2 == 0, -1.0, 1.0).astype(np.float32)
    ssinT = sinT * sgn[:, None]
    cosP, ssinP = cosT[PERM], ssinT[PERM]
    cos128 = np.vstack([cosP, cosP]).copy()            # [128, T]
    ssin128 = np.vstack([ssinP, ssinP]).copy()
    d_y = _np_dt("b")
    utri = np.triu(np.ones((128, 128), dtype=np.float32)).astype(d_y)
    return cos128, ssin128, utri


def _perm_heads(w):
    """Permute each head's 64 columns of w [C, HPC*HD] by PERM."""
    Cdim = w.shape[0]
    return w.reshape(Cdim, HPC, HD)[:, :, PERM].reshape(Cdim, HPC * HD)


def _core_inputs(x, w_attn, w_proj, t_len=T, dtcfg=None):
    """Build the per-core input maps (values only, keyed by logical name)."""
    d_qkv = d_p = _np_dt("b")
    cos128, ssin128, utri = _host_constants(t_len)
    per_core = []
    for core in range(N_CORES):
        b, g = divmod(core, 4)
        h0 = g * HPC * HD                       # column offset of first head
        wq = _perm_heads(w_attn[:, h0:h0 + HPC * HD])
        wk = _perm_heads(w_attn[:, C + h0:C + h0 + HPC * HD]
                         * np.float32(1.0 / np.sqrt(HD)))
        wvs = w_attn[:, 2 * C + h0:2 * C + h0 + HPC * HD]
        per_core.append(dict(
            xT=np.ascontiguousarray(x[b].T).astype(d_qkv),
            wqk=np.ascontiguousarray(np.concatenate([wq, wk], axis=1)).astype(d_qkv),
            wv=np.ascontiguousarray(wvs).astype(d_qkv),
            wo=np.ascontiguousarray(w_proj[h0:h0 + HPC * HD, :]).astype(d_p),
            cost=cos128, ssin=ssin128, utri=utri,
        ))
    return per_core


def kernel(x, w_attn, w_proj):
    from concourse.bass_utils import run_bass_kernel_spmd

    x = np.asarray(x, dtype=np.float32)
    w_attn = np.asarray(w_attn, dtype=np.float32)
    w_proj = np.asarray(w_proj, dtype=np.float32)

    if "nc" not in _CACHE:
        _CACHE["nc"], _CACHE["names"] = _build(T)
    nc, names = _CACHE["nc"], _CACHE["names"]

    per_core = _core_inputs(x, w_attn, w_proj, T)
    in_maps = [{names[k]: v for k, v in m.items()} for m in per_core]
    r = run_bass_kernel_spmd(nc, in_maps, core_ids=list(range(N_CORES)))

    full = np.zeros((B, T, C), dtype=np.float64)
    for core in range(N_CORES):
        full[core // 4] += r.results[core][names["out"]].astype(np.float64)
    return full.astype(np.float32)
